# revision 31
# baseline (speedup 1.0000x reference)
"""Distributed GQA attention prefill kernel for 8 TRN2 NeuronCores.

Problem: llama-style attention, BSZ=2, SEQ=2048, DIM=4096, 32 Q heads,
8 KV heads, head_dim=128, causal prefill (start_pos=0, caches zero).

Sharding: data-parallel over batch (2) x tensor-parallel over heads (4).
Core c = (b, g) with b = c // 4, g = c % 4 handles batch b, Q heads
8g..8g+7, KV heads 2g..2g+1, and wo rows 1024g..1024(g+1). Each core
emits a partial [2048, 4096] output (bf16); the host sums the 4 TP
partials per batch in fp32. No collectives.

On-chip layout trick: everything is computed in "transposed" layouts so
no activation transpose is ever needed on the PE:
  QT[d, t] = wq.T @ x.T       (lhsT = wq natural, rhs = xT from host)
  KT[d, t] = wk.T @ x.T
  V[t, d]  = x @ wv           (lhsT = xT chunk, rhs = wv natural)
  scoresT[kv, q] = K @ QT     (lhsT = KT tile, rhs = QT tile)
  attn[q, d+1]   = P @ [V|1]  (lhsT = expT tile, rhs = V with ones col
                               -> last column accumulates the softmax
                               denominator for free)
RoPE is applied in rotate-half form: the head_dim of wq/wk is permuted
on the host (even dims first, odd dims second) which leaves all dot
products unchanged; cos/sin arrive transposed [64, t] in fp16.

Optimizations vs the 770us baseline (measures ~731us):
  - All DRAM parameters are host-pre-arranged so every DMA descriptor
    is a contiguous >=1KB run per partition (the old layouts generated
    512B-strided descriptors at ~41ns each).
  - DMA transfers complete in global issue order at ~378GB/s aggregate
    (descriptors of every dma_start fan out over all 16 rings), so all
    loads are emitted in exact need-order: window-0/1 K/V inputs, then
    wq per head. Windows 0-1 run K/V-only first (28us of PE work) so
    the 8MB wq never stalls the first q_unit.
  - QT is kept resident in SBUF (32KB/partition) instead of being
    spilled to DRAM and prefetched back.
  - Causal mask add narrowed to the 128-wide diagonal band (the rest
    of a diagonal score tile is fully visible), cutting DVE work that
    sat on the score->exp critical path.
  - Output partials are written in bf16 (host sums partials in fp32);
    q-tiles processed in order [0,1,2,3] so out-proj strip work is
    always available as PE filler during ScalarE exp phases.
"""

import sys

for p in ("/opt/pypackages", "/opt/trn_rl_repo"):
    if p not in sys.path:
        sys.path.insert(0, p)

import numpy as np
import ml_dtypes

BSZ, SEQ, DIM = 2, 2048, 4096
N_HEADS, N_KV, HD = 32, 8, 128
H_L, KV_L = 8, 2          # per-core local Q heads / KV heads
HL = H_L * HD             # 1024 local head dims
N_CORES = 8
WIN = 256                 # stage-A token window
NW = SEQ // WIN
NK = DIM // 128
NQT = SEQ // 512          # attention q-tiles
NEG = -1e9

_cache = {}


def _build():
    import concourse.mybir as mybir
    import concourse.tile as tile
    from concourse import bacc
    from concourse.masks import make_identity
    from contextlib import ExitStack

    f32 = mybir.dt.float32
    f16 = mybir.dt.float16
    bf16 = mybir.dt.bfloat16
    Exp = mybir.ActivationFunctionType.Exp

    nc = bacc.Bacc()
    # host-pre-arranged layouts (p = SBUF partition, ko = 128-row k chunk)
    xT4 = nc.declare_dram_parameter("xT4", [128, NW, NK, WIN], bf16, isOutput=False)
    wq4 = nc.declare_dram_parameter("wq4", [128, H_L, NK, HD], bf16, isOutput=False)
    wk3 = nc.declare_dram_parameter("wk3", [128, NK, KV_L * HD], bf16, isOutput=False)
    wv3 = nc.declare_dram_parameter("wv3", [128, NK, KV_L * HD], bf16, isOutput=False)
    wo4 = nc.declare_dram_parameter("wo4", [128, DIM // 512, H_L, 512], bf16,
                                    isOutput=False)
    cosT = nc.declare_dram_parameter("cosT", [64, SEQ], f16, isOutput=False)
    sinT = nc.declare_dram_parameter("sinT", [64, SEQ], f16, isOutput=False)
    out = nc.declare_dram_parameter("out", [SEQ, DIM], bf16, isOutput=True)

    with tile.TileContext(nc) as tc, ExitStack() as res:
        ps_big = res.enter_context(tc.tile_pool(name="ps_big", bufs=4, space="PSUM"))
        ps_att = res.enter_context(tc.tile_pool(name="ps_att", bufs=4, space="PSUM"))
        resid = res.enter_context(tc.tile_pool(name="resid", bufs=1))

        # per-window K/V tiles (window = 512 tokens for attention indexing)
        kt_w = [resid.tile([128, KV_L, 512], bf16, tag=f"kt{w}", name=f"kt{w}")
                for w in range(NQT)]
        v_w = [resid.tile([128, 4, KV_L, 130], bf16, tag=f"v{w}",
                          name=f"v{w}") for w in range(NQT)]
        qt_sb = resid.tile([128, H_L, SEQ], bf16, tag="qt")   # Q stays resident
        ident = resid.tile([128, 128], bf16, tag="ident")
        cos_sb = resid.tile([64, SEQ], f16, tag="cos")
        sin_sb = resid.tile([64, SEQ], f16, tag="sin")
        mask_sb = resid.tile([128, 4, 512], f32, tag="mask")

        def rope(ps, dst, t0, tw):
            """dst[0:64]=e*c-o*s ; dst[64:128]=e*s+o*c (e=ps[0:64], o=ps[64:128])."""
            c = cos_sb[:, t0:t0 + tw]
            s = sin_sb[:, t0:t0 + tw]
            t1 = rope_pool.tile([64, WIN], f32, tag="r1", name="r1")[:, :tw]
            t2 = rope_pool.tile([64, WIN], f32, tag="r2", name="r2")[:, :tw]
            nc.vector.tensor_mul(t1, ps[0:64, :tw], c)
            nc.vector.tensor_mul(t2, ps[64:128, :tw], s)
            nc.vector.tensor_sub(dst[0:64, :tw], t1, t2)
            t3 = rope_pool.tile([64, WIN], f32, tag="r1", name="r3")[:, :tw]
            t4 = rope_pool.tile([64, WIN], f32, tag="r2", name="r4")[:, :tw]
            nc.vector.tensor_mul(t3, ps[0:64, :tw], s)
            nc.vector.tensor_mul(t4, ps[64:128, :tw], c)
            nc.vector.tensor_add(dst[64:128, :tw], t3, t4)

        # ---- stage A: Q/K/V projection + RoPE in one x-pass ---------------
        with ExitStack() as sa:
            wq_sb = sa.enter_context(tc.tile_pool(name="wq", bufs=1)).tile(
                [128, H_L, NK, HD], bf16, tag="wq")
            wk_sb = sa.enter_context(tc.tile_pool(name="wk", bufs=1)).tile(
                [128, NK, KV_L * HD], bf16, tag="wk")
            wv_sb = sa.enter_context(tc.tile_pool(name="wv", bufs=1)).tile(
                [128, NK, KV_L * HD], bf16, tag="wv")
            xt_pool = sa.enter_context(tc.tile_pool(name="xt", bufs=2))
            rope_pool = sa.enter_context(tc.tile_pool(name="rope", bufs=2))

            # DMA model (measured): every dma_start's descriptors fan out
            # across all 16 rings (~378GB/s aggregate) and complete in global
            # issue order, regardless of which sequencer issued them. So the
            # bytes must be emitted in exact need-order: window0/1 K/V inputs
            # (needed from ~15us), then wq heads (first needed at ~44us after
            # the 28us K/V-only runway), then everything else.
            xt0 = xt_pool.tile([128, NK, WIN], bf16, tag="xt", name="xt0")
            xt1 = xt_pool.tile([128, NK, WIN], bf16, tag="xt", name="xt1")
            nc.sync.dma_start(out=xt0[:, 0:2], in_=xT4[:, 0, 0:2])
            nc.sync.dma_start(out=wk_sb[:, 0:2], in_=wk3[:, 0:2])
            for lo, hi in ((2, 6), (6, 14), (14, 23), (23, 32)):
                nc.sync.dma_start(out=xt0[:, lo:hi], in_=xT4[:, 0, lo:hi])
                nc.sync.dma_start(out=wk_sb[:, lo:hi], in_=wk3[:, lo:hi])
            nc.sync.dma_start(out=cos_sb[:, 0:512], in_=cosT[:, 0:512])
            nc.sync.dma_start(out=sin_sb[:, 0:512], in_=sinT[:, 0:512])
            for g in range(4):
                ks = slice(g * 8, (g + 1) * 8)
                nc.sync.dma_start(out=wv_sb[:, ks], in_=wv3[:, ks])
            for g in range(4):
                ks = slice(g * 8, (g + 1) * 8)
                nc.sync.dma_start(out=xt1[:, ks], in_=xT4[:, 1, ks])
            make_identity(nc, ident)
            # wq per head in need-order (ko halves, 8KB runs per partition)
            for h in range(H_L):
                nc.sync.dma_start(out=wq_sb[:, h, 0:16], in_=wq4[:, h, 0:16])
                nc.sync.dma_start(out=wq_sb[:, h, 16:32], in_=wq4[:, h, 16:32])
            nc.sync.dma_start(out=cos_sb[:, 512:], in_=cosT[:, 512:])
            nc.sync.dma_start(out=sin_sb[:, 512:], in_=sinT[:, 512:])
            # causal band mask generated on-chip: mask[p, r, q] = 0 if
            # q - p - 128*r >= 0 (visible) else -1e9
            nc.vector.memset(mask_sb, 0.0)
            for r_ in range(4):
                nc.gpsimd.affine_select(
                    out=mask_sb[:, r_, :], in_=mask_sb[:, r_, :],
                    pattern=[[1, 512]], compare_op=mybir.AluOpType.is_ge,
                    fill=NEG, base=-(128 * r_), channel_multiplier=-1)
            for w_ in range(NQT):
                nc.vector.memset(v_w[w_][:, :, :, 128:129], 1.0)

            def kv_unit(xt, w):
                t0 = w * WIN
                wa, wo512 = t0 // 512, (t0 % 512)
                for kh in range(KV_L):
                    ps = ps_big.tile([128, 512], f32, tag="big", name="psk")
                    for k in range(NK):
                        nc.tensor.matmul(
                            ps[:, :WIN], wk_sb[:, k, kh * HD:(kh + 1) * HD],
                            xt[:, k], start=(k == 0), stop=(k == NK - 1))
                    rope(ps, kt_w[wa][:, kh, wo512:wo512 + WIN], t0, WIN)
                for tc_ in range(WIN // 128):
                    ps = ps_big.tile([128, 512], f32, tag="big", name="psv")
                    for k in range(NK):
                        nc.tensor.matmul(
                            ps[:, :KV_L * HD], xt[:, k, tc_ * 128:(tc_ + 1) * 128],
                            wv_sb[:, k], start=(k == 0), stop=(k == NK - 1))
                    for kh in range(KV_L):
                        nc.scalar.copy(
                            v_w[wa][:, wo512 // 128 + tc_, kh, 0:128],
                            ps[:, kh * HD:(kh + 1) * HD])

            def q_unit(xt_, t0_, h):
                ps = ps_big.tile([128, 512], f32, tag="big", name="psq")
                for k in range(NK):
                    nc.tensor.matmul(
                        ps[:, :WIN], wq_sb[:, h, k],
                        xt_[:, k], start=(k == 0), stop=(k == NK - 1))
                rope(ps, qt_sb[:, h, t0_:t0_ + WIN], t0_, WIN)

            def load_xt(w):
                xt = xt_pool.tile([128, NK, WIN], bf16, tag="xt", name="xt")
                for g in range(4):
                    ks = slice(g * 8, (g + 1) * 8)
                    nc.sync.dma_start(out=xt[:, ks], in_=xT4[:, w, ks])
                return xt

            # windows 0-1: K/V only (28us of PE work) so the 8MB of wq can
            # stream in before the first q_unit needs it; the deferred Q
            # work for windows 0-1 then runs back-to-back.
            kv_unit(xt0, 0)
            kv_unit(xt1, 1)
            for h in range(H_L):
                q_unit(xt0, 0, h)
            for h in range(H_L):
                q_unit(xt1, WIN, h)
            for w in range(2, NW):
                xt = load_xt(w)
                kv_unit(xt, w)
                for h in range(H_L):
                    q_unit(xt, w * WIN, h)

        # ---- stage B (attention) with stage C (out-proj) interleaved ------
        with ExitStack() as bc:
            exp_pool = bc.enter_context(tc.tile_pool(name="exp", bufs=16))
            asb_pool = bc.enter_context(tc.tile_pool(name="asb", bufs=16))
            rec_pool = bc.enter_context(tc.tile_pool(name="rec", bufs=16))
            at_sb = bc.enter_context(tc.tile_pool(name="at", bufs=1)).tile(
                [128, H_L, SEQ], bf16, tag="at")
            wo_pool = bc.enter_context(tc.tile_pool(name="wo", bufs=3))
            out_pool = bc.enter_context(tc.tile_pool(name="outp", bufs=8))

            pending = []

            def flush_pending():
                while pending:
                    pending.pop(0)()

            wo_cur = [None]

            def make_strip(qs_):
                """Emission closures for out-proj of token strip qs_ (4 ti)."""
                cls = []
                for di in range(DIM // 512):
                    def load_wo(di=di):
                        wot = wo_pool.tile([128, H_L, 512], bf16, tag="wo",
                                           name="wot")
                        nc.sync.dma_start(out=wot[:, 0:4], in_=wo4[:, di, 0:4])
                        nc.sync.dma_start(out=wot[:, 4:8], in_=wo4[:, di, 4:8])
                        wo_cur[0] = wot
                    cls.append(load_wo)
                    for tj in range(4):
                        def pair(di=di, ti=qs_ * 4 + tj):
                            wot = wo_cur[0]
                            # ps_att: don't contend with exp-pending score
                            # tiles in ps_big while ACT drains them
                            ps = ps_att.tile([128, 512], f32, tag="att",
                                             name="pso")
                            for ho in range(H_L):
                                nc.tensor.matmul(
                                    ps, at_sb[:, ho, ti * 128:(ti + 1) * 128],
                                    wot[:, ho], start=(ho == 0),
                                    stop=(ho == H_L - 1))
                            osb = out_pool.tile([128, 512], bf16, tag="osb",
                                                name="osb")
                            nc.vector.tensor_copy(osb, ps)
                            # alternate HWDGE sequencers so neither backs up
                            eng = nc.sync if (ti + di) % 2 else nc.scalar
                            eng.dma_start(
                                out=out[ti * 128:(ti + 1) * 128,
                                        di * 512:(di + 1) * 512],
                                in_=osb)
                        cls.append(pair)
                return cls

            cqueue = []

            # q0 first: it is the smallest tile (fewest exp tiles per head),
            # so running it filler-less costs the least, and its strip then
            # feeds PE fillers during every later q-tile.
            order = [0, 1, 2, 3]

            for idx, qi in enumerate(order):
                q0 = qi * 512
                if idx >= 1:
                    cqueue.extend(make_strip(order[idx - 1]))
                for h in range(H_L):
                    kh = h // 4
                    nkv = 4 * (qi + 1)
                    pes = []
                    for kvt in range(nkv):
                        # columns q < r*128 of a diagonal tile are fully masked
                        r = kvt - 4 * qi
                        c0 = max(r, 0) * 128
                        ps = ps_big.tile([128, 512], f32, tag="big", name="pss")
                        nc.tensor.matmul(
                            ps[:, c0:], kt_w[kvt // 4][:, kh,
                                               (kvt % 4) * 128:(kvt % 4 + 1) * 128],
                            qt_sb[:, h, q0 + c0:q0 + 512], start=True, stop=True)
                        if r >= 0:
                            # only the 128-wide diagonal band is ever masked:
                            # cols >= c0+128 of a diag tile are fully visible
                            nc.vector.tensor_add(ps[:, c0:c0 + 128],
                                                 ps[:, c0:c0 + 128],
                                                 mask_sb[:, r, c0:c0 + 128])
                        pe = exp_pool.tile([128, 512], bf16, tag="exp", name="pe")
                        nc.scalar.activation(pe[:, c0:], ps[:, c0:], Exp)
                        pes.append(pe)
                        if kvt == 3:
                            flush_pending()
                    # PE filler while ScalarE computes this head's exps
                    for _ in range(5):
                        if cqueue:
                            cqueue.pop(0)()
                    flush_pending()
                    aps = [ps_att.tile([128, 129], f32, tag="att", name=f"att{_qc}")
                           for _qc in range(4)]
                    for kvt in range(nkv):
                        for qc in range(4):
                            if qc < kvt - 4 * qi:
                                continue  # q-chunk entirely masked for this kv
                            nc.tensor.matmul(
                                aps[qc], pes[kvt][:, qc * 128:(qc + 1) * 128],
                                v_w[kvt // 4][:, kvt % 4, kh, 0:129],
                                start=(kvt == 0), stop=(kvt == 4 * qi + qc))
                    asbs = []
                    for qc in range(4):
                        rec = rec_pool.tile([128, 1], f32, tag="rec", name="rec")
                        nc.vector.reciprocal(rec, aps[qc][:, 128:129])
                        asb = asb_pool.tile([128, 128], bf16, tag="asb", name="asb")
                        nc.vector.tensor_scalar_mul(asb, aps[qc][:, 0:128], rec)
                        asbs.append(asb)

                    def defer(h=h, q0=q0, asbs=asbs):
                        for qc in range(4):
                            pst = ps_att.tile([128, 128], bf16, tag="att",
                                              name="pst")
                            nc.tensor.transpose(pst, asbs[qc], ident)
                            nc.vector.tensor_copy(
                                at_sb[:, h, q0 + qc * 128:q0 + (qc + 1) * 128],
                                pst)
                    pending.append(defer)
                # the qi-end flush is deferred into the next qi's first-head
                # kvt==3 flush so the transposes overlap fresh score work
                for c in cqueue:
                    c()
                cqueue = []
            flush_pending()
            for c in make_strip(order[-1]):
                c()

    nc.finalize()
    return nc


def _prep_inputs(x, wq, wk, wv, wo, freqs_cos, freqs_sin):
    """Host-side shard prep. Returns in_maps for cores 0..7."""
    bf = ml_dtypes.bfloat16
    perm = np.concatenate([np.arange(0, HD, 2), np.arange(1, HD, 2)])  # rotate-half

    wq_p = (wq.astype(np.float32) / np.sqrt(HD)).reshape(DIM, N_HEADS, HD)[:, :, perm]
    wk_p = wk.astype(np.float32).reshape(DIM, N_KV, HD)[:, :, perm]

    cosT = np.ascontiguousarray(freqs_cos.astype(np.float16).T)  # [64, SEQ]
    sinT = np.ascontiguousarray(freqs_sin.astype(np.float16).T)

    # x[b].T -> [ko, p, w, t] -> [p, w, ko, t] so each window DMA is one
    # contiguous 16KB run per partition
    xT4s = []
    for b in range(BSZ):
        xb = np.ascontiguousarray(x[b].astype(np.float32).T).astype(bf)
        xT4s.append(np.ascontiguousarray(
            xb.reshape(NK, 128, NW, WIN).transpose(1, 2, 0, 3)))

    in_maps = []
    for c in range(N_CORES):
        b, g = c // 4, c % 4
        wq_g = wq_p[:, g * H_L:(g + 1) * H_L]              # [DIM, 8, 128]
        wq4 = np.ascontiguousarray(
            wq_g.reshape(NK, 128, H_L, HD).transpose(1, 2, 0, 3)).astype(bf)
        wk_g = wk_p[:, g * KV_L:(g + 1) * KV_L].reshape(DIM, KV_L * HD)
        wk3 = np.ascontiguousarray(
            wk_g.reshape(NK, 128, KV_L * HD).transpose(1, 0, 2)).astype(bf)
        wv_g = wv[:, g * KV_L * HD:(g + 1) * KV_L * HD].astype(np.float32)
        wv3 = np.ascontiguousarray(
            wv_g.reshape(NK, 128, KV_L * HD).transpose(1, 0, 2)).astype(bf)
        wo_g = wo[g * HL:(g + 1) * HL].astype(np.float32)  # [1024, 4096]
        wo4 = np.ascontiguousarray(
            wo_g.reshape(H_L, 128, DIM // 512, 512).transpose(1, 2, 0, 3)).astype(bf)
        in_maps.append({
            "xT4": xT4s[b], "wq4": wq4, "wk3": wk3, "wv3": wv3, "wo4": wo4,
            "cosT": cosT, "sinT": sinT,
        })
    return in_maps


def _run(inputs, trace=False):
    from concourse.bass_utils import run_bass_kernel_spmd

    if "nc" not in _cache:
        _cache["nc"] = _build()
    nc = _cache["nc"]

    in_maps = _prep_inputs(
        np.asarray(inputs["x"]), np.asarray(inputs["wq"]),
        np.asarray(inputs["wk"]), np.asarray(inputs["wv"]),
        np.asarray(inputs["wo"]), np.asarray(inputs["freqs_cos"]),
        np.asarray(inputs["freqs_sin"]))

    res = run_bass_kernel_spmd(nc, in_maps, core_ids=list(range(N_CORES)),
                               trace=trace)
    out = np.zeros((BSZ, SEQ, DIM), np.float32)
    for c in range(N_CORES):
        out[c // 4] += res.results[c]["out"].astype(np.float32)
    return out, res


def kernel(**inputs) -> np.ndarray:
    out, _ = _run(inputs, trace=False)
    return out


# revision 32
# speedup vs baseline: 1.0002x; 1.0002x over previous
"""Distributed GQA attention prefill kernel for 8 TRN2 NeuronCores.

Problem: llama-style attention, BSZ=2, SEQ=2048, DIM=4096, 32 Q heads,
8 KV heads, head_dim=128, causal prefill (start_pos=0, caches zero).

Sharding: data-parallel over batch (2) x tensor-parallel over heads (4).
Core c = (b, g) with b = c // 4, g = c % 4 handles batch b, Q heads
8g..8g+7, KV heads 2g..2g+1, and wo rows 1024g..1024(g+1). Each core
emits a partial [2048, 4096] output (bf16); the host sums the 4 TP
partials per batch in fp32. No collectives.

On-chip layout trick: everything is computed in "transposed" layouts so
no activation transpose is ever needed on the PE:
  QT[d, t] = wq.T @ x.T       (lhsT = wq natural, rhs = xT from host)
  KT[d, t] = wk.T @ x.T
  V[t, d]  = x @ wv           (lhsT = xT chunk, rhs = wv natural)
  scoresT[kv, q] = K @ QT     (lhsT = KT tile, rhs = QT tile)
  attn[q, d+1]   = P @ [V|1]  (lhsT = expT tile, rhs = V with ones col
                               -> last column accumulates the softmax
                               denominator for free)
RoPE is applied in rotate-half form: the head_dim of wq/wk is permuted
on the host (even dims first, odd dims second) which leaves all dot
products unchanged; cos/sin arrive transposed [64, t] in fp16.

Optimizations vs the 770us baseline (measures ~731us):
  - All DRAM parameters are host-pre-arranged so every DMA descriptor
    is a contiguous >=1KB run per partition (the old layouts generated
    512B-strided descriptors at ~41ns each).
  - DMA transfers complete in global issue order at ~378GB/s aggregate
    (descriptors of every dma_start fan out over all 16 rings), so all
    loads are emitted in exact need-order: window-0/1 K/V inputs, then
    wq per head. Windows 0-1 run K/V-only first (28us of PE work) so
    the 8MB wq never stalls the first q_unit.
  - QT is kept resident in SBUF (32KB/partition) instead of being
    spilled to DRAM and prefetched back.
  - Causal mask add narrowed to the 128-wide diagonal band (the rest
    of a diagonal score tile is fully visible), cutting DVE work that
    sat on the score->exp critical path.
  - Output partials are written in bf16 (host sums partials in fp32);
    q-tiles processed in order [0,1,2,3] so out-proj strip work is
    always available as PE filler during ScalarE exp phases.
"""

import sys

for p in ("/opt/pypackages", "/opt/trn_rl_repo"):
    if p not in sys.path:
        sys.path.insert(0, p)

import numpy as np
import ml_dtypes

BSZ, SEQ, DIM = 2, 2048, 4096
N_HEADS, N_KV, HD = 32, 8, 128
H_L, KV_L = 8, 2          # per-core local Q heads / KV heads
HL = H_L * HD             # 1024 local head dims
N_CORES = 8
WIN = 256                 # stage-A token window
NW = SEQ // WIN
NK = DIM // 128
NQT = SEQ // 512          # attention q-tiles
NEG = -1e9

_cache = {}


def _build():
    import concourse.mybir as mybir
    import concourse.tile as tile
    from concourse import bacc
    from concourse.masks import make_identity
    from contextlib import ExitStack

    f32 = mybir.dt.float32
    f16 = mybir.dt.float16
    bf16 = mybir.dt.bfloat16
    Exp = mybir.ActivationFunctionType.Exp

    nc = bacc.Bacc()
    # host-pre-arranged layouts (p = SBUF partition, ko = 128-row k chunk)
    xT4 = nc.declare_dram_parameter("xT4", [128, NW, NK, WIN], bf16, isOutput=False)
    wq4 = nc.declare_dram_parameter("wq4", [128, H_L, NK, HD], bf16, isOutput=False)
    wk3 = nc.declare_dram_parameter("wk3", [128, NK, KV_L * HD], bf16, isOutput=False)
    wv3 = nc.declare_dram_parameter("wv3", [128, NK, KV_L * HD], bf16, isOutput=False)
    wo4 = nc.declare_dram_parameter("wo4", [128, DIM // 512, H_L, 512], bf16,
                                    isOutput=False)
    cosT = nc.declare_dram_parameter("cosT", [64, SEQ], f16, isOutput=False)
    sinT = nc.declare_dram_parameter("sinT", [64, SEQ], f16, isOutput=False)
    out = nc.declare_dram_parameter("out", [SEQ, DIM], bf16, isOutput=True)

    with tile.TileContext(nc) as tc, ExitStack() as res:
        ps_big = res.enter_context(tc.tile_pool(name="ps_big", bufs=4, space="PSUM"))
        ps_att = res.enter_context(tc.tile_pool(name="ps_att", bufs=4, space="PSUM"))
        resid = res.enter_context(tc.tile_pool(name="resid", bufs=1))

        # per-window K/V tiles (window = 512 tokens for attention indexing)
        kt_w = [resid.tile([128, KV_L, 512], bf16, tag=f"kt{w}", name=f"kt{w}")
                for w in range(NQT)]
        v_w = [resid.tile([128, 4, KV_L, 130], bf16, tag=f"v{w}",
                          name=f"v{w}") for w in range(NQT)]
        qt_sb = resid.tile([128, H_L, SEQ], bf16, tag="qt")   # Q stays resident
        ident = resid.tile([128, 128], bf16, tag="ident")
        cos_sb = resid.tile([64, SEQ], f16, tag="cos")
        sin_sb = resid.tile([64, SEQ], f16, tag="sin")
        mask_sb = resid.tile([128, 4, 512], f32, tag="mask")

        def rope(ps, dst, t0, tw):
            """dst[0:64]=e*c-o*s ; dst[64:128]=e*s+o*c (e=ps[0:64], o=ps[64:128])."""
            c = cos_sb[:, t0:t0 + tw]
            s = sin_sb[:, t0:t0 + tw]
            t1 = rope_pool.tile([64, WIN], f32, tag="r1", name="r1")[:, :tw]
            t2 = rope_pool.tile([64, WIN], f32, tag="r2", name="r2")[:, :tw]
            nc.vector.tensor_mul(t1, ps[0:64, :tw], c)
            nc.vector.tensor_mul(t2, ps[64:128, :tw], s)
            nc.vector.tensor_sub(dst[0:64, :tw], t1, t2)
            t3 = rope_pool.tile([64, WIN], f32, tag="r1", name="r3")[:, :tw]
            t4 = rope_pool.tile([64, WIN], f32, tag="r2", name="r4")[:, :tw]
            nc.vector.tensor_mul(t3, ps[0:64, :tw], s)
            nc.vector.tensor_mul(t4, ps[64:128, :tw], c)
            nc.vector.tensor_add(dst[64:128, :tw], t3, t4)

        # ---- stage A: Q/K/V projection + RoPE in one x-pass ---------------
        with ExitStack() as sa:
            wq_sb = sa.enter_context(tc.tile_pool(name="wq", bufs=1)).tile(
                [128, H_L, NK, HD], bf16, tag="wq")
            wk_sb = sa.enter_context(tc.tile_pool(name="wk", bufs=1)).tile(
                [128, NK, KV_L * HD], bf16, tag="wk")
            wv_sb = sa.enter_context(tc.tile_pool(name="wv", bufs=1)).tile(
                [128, NK, KV_L * HD], bf16, tag="wv")
            xt_pool = sa.enter_context(tc.tile_pool(name="xt", bufs=2))
            rope_pool = sa.enter_context(tc.tile_pool(name="rope", bufs=2))

            # DMA model (measured): every dma_start's descriptors fan out
            # across all 16 rings (~378GB/s aggregate) and complete in global
            # issue order, regardless of which sequencer issued them. So the
            # bytes must be emitted in exact need-order: window0/1 K/V inputs
            # (needed from ~15us), then wq heads (first needed at ~44us after
            # the 28us K/V-only runway), then everything else.
            xt0 = xt_pool.tile([128, NK, WIN], bf16, tag="xt", name="xt0")
            xt1 = xt_pool.tile([128, NK, WIN], bf16, tag="xt", name="xt1")
            nc.sync.dma_start(out=xt0[:, 0:2], in_=xT4[:, 0, 0:2])
            nc.sync.dma_start(out=wk_sb[:, 0:2], in_=wk3[:, 0:2])
            for lo, hi in ((2, 6), (6, 14), (14, 23), (23, 32)):
                nc.sync.dma_start(out=xt0[:, lo:hi], in_=xT4[:, 0, lo:hi])
                nc.sync.dma_start(out=wk_sb[:, lo:hi], in_=wk3[:, lo:hi])
            nc.sync.dma_start(out=cos_sb[:, 0:512], in_=cosT[:, 0:512])
            nc.sync.dma_start(out=sin_sb[:, 0:512], in_=sinT[:, 0:512])
            for g in range(4):
                ks = slice(g * 8, (g + 1) * 8)
                nc.sync.dma_start(out=wv_sb[:, ks], in_=wv3[:, ks])
            for g in range(4):
                ks = slice(g * 8, (g + 1) * 8)
                nc.sync.dma_start(out=xt1[:, ks], in_=xT4[:, 1, ks])
            make_identity(nc, ident)
            # PE warm-up: ~4.3us of dummy matmuls (no DMA dependency) so the
            # HAM clock gate opens (needs ~3.4us of sustained PE activity)
            # before the first real matmul's data lands at ~12us — the real
            # chains then run at 2.4GHz instead of a cold 1.2GHz start.
            ps_warm = ps_big.tile([128, 512], f32, tag="big", name="warm")
            for wi in range(40):
                nc.tensor.matmul(ps_warm[:, (wi % 2) * 128:(wi % 2) * 128 + 128],
                                 ident, ident, start=True, stop=True)
            # wq per head in need-order (ko halves, 8KB runs per partition)
            for h in range(H_L):
                nc.sync.dma_start(out=wq_sb[:, h, 0:16], in_=wq4[:, h, 0:16])
                nc.sync.dma_start(out=wq_sb[:, h, 16:32], in_=wq4[:, h, 16:32])
            nc.sync.dma_start(out=cos_sb[:, 512:], in_=cosT[:, 512:])
            nc.sync.dma_start(out=sin_sb[:, 512:], in_=sinT[:, 512:])
            # causal band mask generated on-chip: mask[p, r, q] = 0 if
            # q - p - 128*r >= 0 (visible) else -1e9
            nc.vector.memset(mask_sb, 0.0)
            for r_ in range(4):
                nc.gpsimd.affine_select(
                    out=mask_sb[:, r_, :], in_=mask_sb[:, r_, :],
                    pattern=[[1, 512]], compare_op=mybir.AluOpType.is_ge,
                    fill=NEG, base=-(128 * r_), channel_multiplier=-1)
            for w_ in range(NQT):
                nc.vector.memset(v_w[w_][:, :, :, 128:129], 1.0)

            def kv_unit(xt, w):
                t0 = w * WIN
                wa, wo512 = t0 // 512, (t0 % 512)
                for kh in range(KV_L):
                    ps = ps_big.tile([128, 512], f32, tag="big", name="psk")
                    for k in range(NK):
                        nc.tensor.matmul(
                            ps[:, :WIN], wk_sb[:, k, kh * HD:(kh + 1) * HD],
                            xt[:, k], start=(k == 0), stop=(k == NK - 1))
                    rope(ps, kt_w[wa][:, kh, wo512:wo512 + WIN], t0, WIN)
                for tc_ in range(WIN // 128):
                    ps = ps_big.tile([128, 512], f32, tag="big", name="psv")
                    for k in range(NK):
                        nc.tensor.matmul(
                            ps[:, :KV_L * HD], xt[:, k, tc_ * 128:(tc_ + 1) * 128],
                            wv_sb[:, k], start=(k == 0), stop=(k == NK - 1))
                    for kh in range(KV_L):
                        nc.scalar.copy(
                            v_w[wa][:, wo512 // 128 + tc_, kh, 0:128],
                            ps[:, kh * HD:(kh + 1) * HD])

            def q_unit(xt_, t0_, h):
                ps = ps_big.tile([128, 512], f32, tag="big", name="psq")
                for k in range(NK):
                    nc.tensor.matmul(
                        ps[:, :WIN], wq_sb[:, h, k],
                        xt_[:, k], start=(k == 0), stop=(k == NK - 1))
                rope(ps, qt_sb[:, h, t0_:t0_ + WIN], t0_, WIN)

            def load_xt(w):
                xt = xt_pool.tile([128, NK, WIN], bf16, tag="xt", name="xt")
                for g in range(4):
                    ks = slice(g * 8, (g + 1) * 8)
                    nc.sync.dma_start(out=xt[:, ks], in_=xT4[:, w, ks])
                return xt

            # windows 0-1: K/V only (28us of PE work) so the 8MB of wq can
            # stream in before the first q_unit needs it; the deferred Q
            # work for windows 0-1 then runs back-to-back.
            kv_unit(xt0, 0)
            kv_unit(xt1, 1)
            for h in range(H_L):
                q_unit(xt0, 0, h)
            for h in range(H_L):
                q_unit(xt1, WIN, h)
            for w in range(2, NW):
                xt = load_xt(w)
                kv_unit(xt, w)
                for h in range(H_L):
                    q_unit(xt, w * WIN, h)

        # ---- stage B (attention) with stage C (out-proj) interleaved ------
        with ExitStack() as bc:
            exp_pool = bc.enter_context(tc.tile_pool(name="exp", bufs=16))
            asb_pool = bc.enter_context(tc.tile_pool(name="asb", bufs=16))
            rec_pool = bc.enter_context(tc.tile_pool(name="rec", bufs=16))
            at_sb = bc.enter_context(tc.tile_pool(name="at", bufs=1)).tile(
                [128, H_L, SEQ], bf16, tag="at")
            wo_pool = bc.enter_context(tc.tile_pool(name="wo", bufs=3))
            out_pool = bc.enter_context(tc.tile_pool(name="outp", bufs=8))

            pending = []

            def flush_pending():
                while pending:
                    pending.pop(0)()

            wo_cur = [None]

            def make_strip(qs_):
                """Emission closures for out-proj of token strip qs_ (4 ti)."""
                cls = []
                for di in range(DIM // 512):
                    def load_wo(di=di):
                        wot = wo_pool.tile([128, H_L, 512], bf16, tag="wo",
                                           name="wot")
                        nc.sync.dma_start(out=wot[:, 0:4], in_=wo4[:, di, 0:4])
                        nc.sync.dma_start(out=wot[:, 4:8], in_=wo4[:, di, 4:8])
                        wo_cur[0] = wot
                    cls.append(load_wo)
                    for tj in range(4):
                        def pair(di=di, ti=qs_ * 4 + tj):
                            wot = wo_cur[0]
                            # ps_att: don't contend with exp-pending score
                            # tiles in ps_big while ACT drains them
                            ps = ps_att.tile([128, 512], f32, tag="att",
                                             name="pso")
                            for ho in range(H_L):
                                nc.tensor.matmul(
                                    ps, at_sb[:, ho, ti * 128:(ti + 1) * 128],
                                    wot[:, ho], start=(ho == 0),
                                    stop=(ho == H_L - 1))
                            osb = out_pool.tile([128, 512], bf16, tag="osb",
                                                name="osb")
                            nc.vector.tensor_copy(osb, ps)
                            # alternate HWDGE sequencers so neither backs up
                            eng = nc.sync if (ti + di) % 2 else nc.scalar
                            eng.dma_start(
                                out=out[ti * 128:(ti + 1) * 128,
                                        di * 512:(di + 1) * 512],
                                in_=osb)
                        cls.append(pair)
                return cls

            cqueue = []

            # q0 first: it is the smallest tile (fewest exp tiles per head),
            # so running it filler-less costs the least, and its strip then
            # feeds PE fillers during every later q-tile.
            order = [0, 1, 2, 3]

            for idx, qi in enumerate(order):
                q0 = qi * 512
                if idx >= 1:
                    cqueue.extend(make_strip(order[idx - 1]))
                for h in range(H_L):
                    kh = h // 4
                    nkv = 4 * (qi + 1)
                    pes = []
                    for kvt in range(nkv):
                        # columns q < r*128 of a diagonal tile are fully masked
                        r = kvt - 4 * qi
                        c0 = max(r, 0) * 128
                        ps = ps_big.tile([128, 512], f32, tag="big", name="pss")
                        nc.tensor.matmul(
                            ps[:, c0:], kt_w[kvt // 4][:, kh,
                                               (kvt % 4) * 128:(kvt % 4 + 1) * 128],
                            qt_sb[:, h, q0 + c0:q0 + 512], start=True, stop=True)
                        if r >= 0:
                            # only the 128-wide diagonal band is ever masked:
                            # cols >= c0+128 of a diag tile are fully visible
                            nc.vector.tensor_add(ps[:, c0:c0 + 128],
                                                 ps[:, c0:c0 + 128],
                                                 mask_sb[:, r, c0:c0 + 128])
                        pe = exp_pool.tile([128, 512], bf16, tag="exp", name="pe")
                        nc.scalar.activation(pe[:, c0:], ps[:, c0:], Exp)
                        pes.append(pe)
                        if kvt == 3:
                            flush_pending()
                    # PE filler while ScalarE computes this head's exps
                    for _ in range(5):
                        if cqueue:
                            cqueue.pop(0)()
                    flush_pending()
                    aps = [ps_att.tile([128, 129], f32, tag="att", name=f"att{_qc}")
                           for _qc in range(4)]
                    for kvt in range(nkv):
                        for qc in range(4):
                            if qc < kvt - 4 * qi:
                                continue  # q-chunk entirely masked for this kv
                            nc.tensor.matmul(
                                aps[qc], pes[kvt][:, qc * 128:(qc + 1) * 128],
                                v_w[kvt // 4][:, kvt % 4, kh, 0:129],
                                start=(kvt == 0), stop=(kvt == 4 * qi + qc))
                    asbs = []
                    for qc in range(4):
                        rec = rec_pool.tile([128, 1], f32, tag="rec", name="rec")
                        nc.vector.reciprocal(rec, aps[qc][:, 128:129])
                        asb = asb_pool.tile([128, 128], bf16, tag="asb", name="asb")
                        nc.vector.tensor_scalar_mul(asb, aps[qc][:, 0:128], rec)
                        asbs.append(asb)

                    def defer(h=h, q0=q0, asbs=asbs):
                        for qc in range(4):
                            pst = ps_att.tile([128, 128], bf16, tag="att",
                                              name="pst")
                            nc.tensor.transpose(pst, asbs[qc], ident)
                            nc.vector.tensor_copy(
                                at_sb[:, h, q0 + qc * 128:q0 + (qc + 1) * 128],
                                pst)
                    pending.append(defer)
                # the qi-end flush is deferred into the next qi's first-head
                # kvt==3 flush so the transposes overlap fresh score work
                for c in cqueue:
                    c()
                cqueue = []
            flush_pending()
            for c in make_strip(order[-1]):
                c()

    nc.finalize()
    return nc


def _prep_inputs(x, wq, wk, wv, wo, freqs_cos, freqs_sin):
    """Host-side shard prep. Returns in_maps for cores 0..7."""
    bf = ml_dtypes.bfloat16
    perm = np.concatenate([np.arange(0, HD, 2), np.arange(1, HD, 2)])  # rotate-half

    wq_p = (wq.astype(np.float32) / np.sqrt(HD)).reshape(DIM, N_HEADS, HD)[:, :, perm]
    wk_p = wk.astype(np.float32).reshape(DIM, N_KV, HD)[:, :, perm]

    cosT = np.ascontiguousarray(freqs_cos.astype(np.float16).T)  # [64, SEQ]
    sinT = np.ascontiguousarray(freqs_sin.astype(np.float16).T)

    # x[b].T -> [ko, p, w, t] -> [p, w, ko, t] so each window DMA is one
    # contiguous 16KB run per partition
    xT4s = []
    for b in range(BSZ):
        xb = np.ascontiguousarray(x[b].astype(np.float32).T).astype(bf)
        xT4s.append(np.ascontiguousarray(
            xb.reshape(NK, 128, NW, WIN).transpose(1, 2, 0, 3)))

    in_maps = []
    for c in range(N_CORES):
        b, g = c // 4, c % 4
        wq_g = wq_p[:, g * H_L:(g + 1) * H_L]              # [DIM, 8, 128]
        wq4 = np.ascontiguousarray(
            wq_g.reshape(NK, 128, H_L, HD).transpose(1, 2, 0, 3)).astype(bf)
        wk_g = wk_p[:, g * KV_L:(g + 1) * KV_L].reshape(DIM, KV_L * HD)
        wk3 = np.ascontiguousarray(
            wk_g.reshape(NK, 128, KV_L * HD).transpose(1, 0, 2)).astype(bf)
        wv_g = wv[:, g * KV_L * HD:(g + 1) * KV_L * HD].astype(np.float32)
        wv3 = np.ascontiguousarray(
            wv_g.reshape(NK, 128, KV_L * HD).transpose(1, 0, 2)).astype(bf)
        wo_g = wo[g * HL:(g + 1) * HL].astype(np.float32)  # [1024, 4096]
        wo4 = np.ascontiguousarray(
            wo_g.reshape(H_L, 128, DIM // 512, 512).transpose(1, 2, 0, 3)).astype(bf)
        in_maps.append({
            "xT4": xT4s[b], "wq4": wq4, "wk3": wk3, "wv3": wv3, "wo4": wo4,
            "cosT": cosT, "sinT": sinT,
        })
    return in_maps


def _run(inputs, trace=False):
    from concourse.bass_utils import run_bass_kernel_spmd

    if "nc" not in _cache:
        _cache["nc"] = _build()
    nc = _cache["nc"]

    in_maps = _prep_inputs(
        np.asarray(inputs["x"]), np.asarray(inputs["wq"]),
        np.asarray(inputs["wk"]), np.asarray(inputs["wv"]),
        np.asarray(inputs["wo"]), np.asarray(inputs["freqs_cos"]),
        np.asarray(inputs["freqs_sin"]))

    res = run_bass_kernel_spmd(nc, in_maps, core_ids=list(range(N_CORES)),
                               trace=trace)
    out = np.zeros((BSZ, SEQ, DIM), np.float32)
    for c in range(N_CORES):
        out[c // 4] += res.results[c]["out"].astype(np.float32)
    return out, res


def kernel(**inputs) -> np.ndarray:
    out, _ = _run(inputs, trace=False)
    return out
